# revision 1
# baseline (speedup 1.0000x reference)
"""GCNConv (PyG semantics: normalize=True, add_self_loops=True, edge_weight)
as a Trainium2 Bass kernel, SPMD over 8 NeuronCores.

Strategy: shard destination nodes across the 8 cores; within a core, process
one 128-dst block at a time. The normalized adjacency (self loops included,
A[dst, src] = dinv[src]*w*dinv[dst]) is sparse: a 128-dst block receives
only ~2.1k edges, so instead of sweeping all ~10k source columns (79
k-tiles) per block, the host packs each block's distinct source rows into
dense k-tiles and emits a matching compacted A^T stripe. Destinations are
re-blocked by a greedy shared-source clustering (_block_assign) before
packing, which drops distinct sources per block from ~1956 to ~1600 ->
KT=13 k-tiles. Per block the PE does KT matmuls [128x128 bf16 A'] x
[128x512 fp8e3 x'] accumulating agg in PSUM — a 6x FLOP reduction over the
dense sweep at identical numerics on the A side.

The packed x rows are quantized to fp8 e3m4 at 2x scale (4 mantissa bits;
the 1/2 is folded into A'), which keeps the whole-graph working set small
enough (~16MB/core) that both the x stream and the A^T stripes are loaded
into SBUF once in a pipelined prologue and reused; steady state moves only
the output. A' stays bf16 — simulated end-to-end rel-err is 1.3e-2 vs the
2e-2 gate (fp8 on both operands fails). The transform out = agg @ W + b runs
one block behind the sweep in bf16: agg.T is formed by plain matmuls against
identity (cheaper than PE transpose-mode: the 128-col LDWEIGHTS overlaps the
previous matmul), ACT rounds PSUM->SBUF, the W matmuls run at full PE rate,
and DVE adds the bias into a bf16 accumulator that leaves in two DMAs.

All index preprocessing/packing/clustering is host-side; every
O(E*D)/O(N*D^2) FLOP runs on device. Per core per pass the PE streams 9,216
moving columns per block (sweep 13x512 + agg.T 4x128 + W 4x512) = 38.4us at
2.4GHz; measured ~38us, i.e. ~the column-streaming floor, with only ~1.3MB
of steady-state DMA (fully hidden). fp8e4 DoubleRow (the only faster PE
mode) fails the 2e-2 gate even when restricted to low-magnitude edges, and
KT=12 needs a tighter clustering than the expander graph admits, so this is
the floor for this accuracy target.
"""
from contextlib import ExitStack

import numpy as np
import ml_dtypes

import concourse.bacc as bacc
import concourse.mybir as mybir
import concourse.tile as tile
from concourse.bass_utils import run_bass_kernel_spmd

P = 128
CORES = 8
BF16 = mybir.dt.bfloat16
F32 = mybir.dt.float32
FP8E3 = mybir.dt.float8e3


def _block_assign(n, nblocks, ss, bounds, cap):
    """Greedy hypergraph-style clustering: dsts (ascending degree) are
    placed into the block where they add the fewest new distinct sources,
    subject to the distinct-count cap. On the expander-random graph this
    compacts every 128-dst block from ~1956 distinct sources down to
    ~1600, cutting the sweep from 16 to 13 k-tiles per block."""
    degs = bounds[1:] - bounds[:-1]
    masks = np.zeros((nblocks, n), bool)
    counts = np.zeros(nblocks, np.int64)
    dist = np.zeros(nblocks, np.int64)
    assign = np.empty(n, np.int64)
    for d in np.argsort(degs, kind="stable"):
        cols = ss[bounds[d]:bounds[d + 1]]
        adds = (~masks[:, cols]).sum(axis=1)
        res = dist + adds
        ok = counts < P
        under = ok & (res <= cap)
        if under.any():
            pool = np.where(under)[0]
            g = int(pool[np.lexsort((counts[pool], adds[pool]))[0]])
        else:
            pool = np.where(ok)[0]
            g = int(pool[np.argmin(res[pool])])
        masks[g, cols] = True
        counts[g] += 1
        dist[g] += adds[g]
        assign[d] = g
    return assign


def _preprocess(x, edge_index, edge_attr):
    """Self loops, symmetric normalization, then clustered per-block source
    packing: dst nodes are re-blocked to maximize shared sources (see
    _block_assign) and each block's distinct sources are compacted into KT
    k-tiles. Outputs the packed fp8 x stream, the compacted bf16 A^T
    stripes in SBUF-swizzled layout, and the dst->output-row permutation."""
    x = np.asarray(x, np.float32)
    n, d_in = x.shape
    src = np.asarray(edge_index[0], np.int64)
    dst = np.asarray(edge_index[1], np.int64)
    loop = np.arange(n, dtype=np.int64)
    src_f = np.concatenate([src, loop])
    dst_f = np.concatenate([dst, loop])
    ew = np.concatenate(
        [np.asarray(edge_attr, np.float64), np.ones(n, np.float64)])

    deg = np.zeros(n, np.float64)
    np.add.at(deg, dst_f, ew)
    dinv = np.where(deg > 0, 1.0 / np.sqrt(np.maximum(deg, 1e-300)), 0.0)
    sc = (dinv[src_f] * ew * dinv[dst_f]).astype(np.float32)

    bpc = -(-n // (CORES * P))           # dst blocks per core
    nblocks = CORES * bpc

    # per-dst edge lists (sorted by dst) drive the clustering
    eorder = np.argsort(dst_f, kind="stable")
    ds, ss2 = dst_f[eorder], src_f[eorder]
    dbounds = np.searchsorted(ds, np.arange(n + 1))
    assign = _block_assign(n, nblocks, ss2, dbounds, cap=13 * P)
    # lane within block, and the dst -> flat output row map for unsharding
    lane = np.zeros(n, np.int64)
    for b in range(nblocks):
        members = np.where(assign == b)[0]
        lane[members] = np.arange(len(members))
    row_of = assign * P + lane

    blk = assign[dst_f]
    m_of = lane[dst_f]

    order = np.argsort(blk, kind="stable")
    blk_s, src_s, m_s, sc_s = blk[order], src_f[order], m_of[order], sc[order]
    bounds = np.searchsorted(blk_s, np.arange(nblocks + 1))
    uniq = []
    slot = np.empty(len(src_s), np.int64)
    for b in range(nblocks):
        lo, hi = bounds[b], bounds[b + 1]
        u, inv = np.unique(src_s[lo:hi], return_inverse=True)
        uniq.append(u)
        slot[lo:hi] = inv
    kt = max(1, max(-(-len(u) // P) for u in uniq))

    at = np.zeros((nblocks, P, kt * P), np.float32)
    np.add.at(at, (blk_s, slot % P, (slot // P) * P + m_s), sc_s)
    at = (at * 0.5).astype(ml_dtypes.bfloat16)     # x carries a 2x scale
    at = at.reshape(CORES, bpc, P, kt * P)

    x2q = (x * 2.0).astype(ml_dtypes.float8_e3m4)
    xq = np.zeros((nblocks, kt * P, d_in), ml_dtypes.float8_e3m4)
    for b in range(nblocks):
        u = uniq[b]
        xq[b, :len(u)] = x2q[u]
    # swizzle to SBUF layout: [p, k*d_in + d] = row k*P+p of the packed block
    xq = (xq.reshape(nblocks, kt, P, d_in).transpose(0, 2, 1, 3)
          .reshape(CORES, bpc, P, kt * d_in))

    return dict(bpc=bpc, kt=kt, at=at, xq=xq, row_of=row_of)


def _build_module(n, d_in, d_out, bpc, kt, reps=1):
    """Emit the SPMD per-core Bass program."""
    assert d_in % P == 0 and d_out % P == 0
    kt_w = d_in // P

    nc = bacc.Bacc("TRN2", target_bir_lowering=False, debug=False)
    xq_d = nc.dram_tensor("xq", [bpc, P, kt * d_in], FP8E3,
                          kind="ExternalInput")
    at_d = nc.dram_tensor("at", [bpc, P, kt * P], BF16, kind="ExternalInput")
    W_d = nc.dram_tensor("W", [P, kt_w * d_out], BF16, kind="ExternalInput")
    bias_d = nc.dram_tensor("bias", [P, d_out], F32, kind="ExternalInput")
    ident_d = nc.dram_tensor("ident", [P, P], BF16, kind="ExternalInput")
    out_d = nc.dram_tensor("out", [bpc, P, d_out], BF16,
                           kind="ExternalOutput")

    with tile.TileContext(nc) as tc, ExitStack() as ctx:
        const = ctx.enter_context(tc.tile_pool(name="const", bufs=1))
        apool = ctx.enter_context(tc.tile_pool(name="aggsb", bufs=2))
        tpool = ctx.enter_context(tc.tile_pool(name="atsb", bufs=8))
        ps_agg = ctx.enter_context(tc.tile_pool(name="ps_agg", bufs=2,
                                                space="PSUM"))
        ps_t = ctx.enter_context(tc.tile_pool(name="ps_t", bufs=4,
                                              space="PSUM"))
        ps_out = ctx.enter_context(tc.tile_pool(name="ps_out", bufs=2,
                                                space="PSUM"))

        # the whole per-core working set (packed x + A^T stripes) lives in
        # SBUF: loaded once here, consumed by the block loop as each DMA
        # lands (first pass streams through, later reps reuse)
        W_sb = const.tile([P, kt_w, d_out], BF16)
        nc.scalar.dma_start(W_sb[:], W_d.ap().rearrange("p (k d) -> p k d",
                                                        d=d_out))
        bias_sb = const.tile([P, d_out], F32)
        nc.scalar.dma_start(bias_sb[:], bias_d[:, :])
        ident_sb = const.tile([P, P], BF16)
        nc.scalar.dma_start(ident_sb[:], ident_d[:, :])
        out_acc = const.tile([P, bpc, d_out], BF16)
        at_tiles, xq_tiles = [], []
        for g in range(bpc):
            a = const.tile([P, kt, P], BF16, tag=f"at{g}")
            nc.scalar.dma_start(a[:], at_d[g].rearrange("p (k m) -> p k m",
                                                        m=P))
            at_tiles.append(a)
            xx = const.tile([P, kt, d_in], FP8E3, tag=f"xq{g}")
            nc.sync.dma_start(xx[:], xq_d[g].rearrange("p (k d) -> p k d",
                                                       d=d_in))
            xq_tiles.append(xx)

        def transform(g, agg_ps):
            # agg_ps [P dst, d_in] fp32 PSUM -> out_acc[:, g, :] = agg@W + b.
            # agg.T via plain matmuls against identity-free weight loads:
            # lhsT=agg_slice, rhs=W row-block would need agg.T as stationary,
            # so form agg.T slices first, all four ahead of the W matmuls to
            # keep the in-order PE queue from stalling on the ACT relays.
            agg_sb = apool.tile([P, d_in], BF16, tag="agg")
            for ki in range(kt_w):
                # chunked so the first agg.T matmul only waits ~one chunk
                nc.scalar.copy(agg_sb[:, ki * P:(ki + 1) * P],
                               agg_ps[:, ki * P:(ki + 1) * P])
            pts, aTs = [], []
            for ki in range(kt_w):
                pt = ps_t.tile([P, P], F32, tag="pt")
                nc.tensor.matmul(pt[:], agg_sb[:, ki * P:(ki + 1) * P],
                                 ident_sb[:], start=True, stop=True)
                pts.append(pt)
            for ki in range(kt_w):
                aT = tpool.tile([P, P], BF16, tag="aT")
                nc.scalar.copy(aT[:], pts[ki][:])
                aTs.append(aT)
            out_ps = ps_out.tile([P, d_out], F32)
            for ki in range(kt_w):
                nc.tensor.matmul(out_ps[:], aTs[ki][:], W_sb[:, ki, :],
                                 start=(ki == 0), stop=(ki == kt_w - 1))
            nc.vector.tensor_add(out_acc[:, g, :], out_ps[:], bias_sb[:])

        order = [g for _ in range(reps) for g in range(bpc)]
        pending = None              # (g, agg_ps) awaiting transform
        for i, g in enumerate(order):
            at_sb, x_sb = at_tiles[g], xq_tiles[g]
            agg_ps = ps_agg.tile([P, d_in], F32)
            # the previous block's transform is issued two matmuls into this
            # block's sweep so its ACT PSUM->SBUF copy is done by the time
            # the PE reaches the agg.T matmuls
            for k in range(kt):
                nc.tensor.matmul(agg_ps[:], at_sb[:, k, :], x_sb[:, k, :],
                                 start=(k == 0), stop=(k == kt - 1))
                if k == 1 and pending is not None:
                    transform(*pending)
                    pending = None
            if pending is not None:
                transform(*pending)
            pending = (g, agg_ps)
            if g == bpc // 2 and g > 0:
                nc.scalar.dma_start(
                    out_d.ap().rearrange("g p d -> p g d")[:, :g, :],
                    out_acc[:, :g, :])
        transform(*pending)
        nc.scalar.dma_start(
            out_d.ap().rearrange("g p d -> p g d")[:, bpc // 2:, :],
            out_acc[:, bpc // 2:, :])

    nc.compile()
    return nc


def _make_in_maps(x, W, b, pre):
    n, d_in = np.asarray(x).shape
    d_out = np.asarray(W).shape[1]
    kt_w = d_in // P
    W16 = np.ascontiguousarray(
        np.asarray(W, np.float32).astype(ml_dtypes.bfloat16)
        .reshape(kt_w, P, d_out).transpose(1, 0, 2).reshape(P, kt_w * d_out))
    bias_bcast = np.ascontiguousarray(
        np.tile(np.asarray(b, np.float32)[None, :], (P, 1)))
    return [
        dict(xq=np.ascontiguousarray(pre["xq"][c]),
             at=np.ascontiguousarray(pre["at"][c]),
             W=W16, bias=bias_bcast,
             ident=np.eye(P, dtype=ml_dtypes.bfloat16))
        for c in range(CORES)
    ]


def kernel(x, edge_index, edge_attr, W, b):
    x = np.asarray(x)
    n, d_in = x.shape
    d_out = np.asarray(W).shape[1]
    pre = _preprocess(x, edge_index, edge_attr)
    nc = _build_module(n, d_in, d_out, pre["bpc"], pre["kt"])
    in_maps = _make_in_maps(x, W, b, pre)
    res = run_bass_kernel_spmd(nc, in_maps, list(range(CORES)))
    out_all = np.concatenate([res.results[c]["out"] for c in range(CORES)],
                             axis=0)            # [CORES*bpc, P, d_out]
    out = out_all.reshape(-1, d_out)[pre["row_of"]]   # undo dst re-blocking
    return np.ascontiguousarray(out.astype(np.float32))

